# revision 35
# baseline (speedup 1.0000x reference)
"""Trainium2 Bass kernel for ConvolutionalAttention2D (linear attention with 1x1 convs).

Reference computation (per batch b):
    q = Wq x ; k = Wk x ; v = Wv x          (1x1 convs == channel matmuls)
    phi(t) = elu(t) + 1
    qv = phi(q) @ phi(v)^T                  ([C, C] context matrix, contract over pixels)
    out = Wo (qv @ phi(k)) + bo

Kernel strategy (8 NeuronCores, data-parallel over batch B=16 -> 2 batches/core):
  - Projections (q,v transposed layout + k natural layout) and the qv
    contraction run as fp8(e4m3) DoubleRow matmuls: contraction dim 256 =
    2 k-tiles processed 2 rows/cycle -> half the PE time of bf16.
  - phi(t) = elu(t)+1 is approximated by a single fused custom DVE op:
        phi(t) ~= min((C1 + C0*t)^8, C2) + relu(t)
    with coefficients fitted end-to-end (rel err ~2e-3 incl. fp8). One
    PSUM pass per phi span instead of ACT-exp + DVE fixup.
  - Most phi_k spans use ACT (Exp+Relu) + DVE min + GPSIMD add ("scheme G")
    to balance load across all four engines.
  - Stage D (out = (Wo qv) @ phi_k) stays bf16 (fp8 W2 overflows/too coarse).
  - bo is added on the host (it's a [C] broadcast; free there).
  - Output written bf16, upcast on host.
  - Execution is a flat pipeline of batch-units: each unit runs one batch's
    B/A window with the previous unit's stage-D spans woven into the middle
    (head-of-line-safe), qv accumulation inline; For_i iterations unroll 8
    bodies to amortize the loop's all-engine barrier.
"""

from contextlib import ExitStack

import numpy as np

import concourse.bacc as bacc
import concourse.tile as tile
from concourse import mybir
from concourse import bass_utils

B, C, H, W = 16, 256, 64, 64
HW = H * W
NCORES = 8
NB = B // NCORES  # batches per core

FP = mybir.dt.float32
BF = mybir.dt.bfloat16
F32R = mybir.dt.float32r
F8 = mybir.dt.float8e4
AF = mybir.ActivationFunctionType
OP = mybir.AluOpType
DR = mybir.MatmulPerfMode.DoubleRow

# phi(t) ~= min((PC1 + PC0*t)^8, PC2) + relu(t), coefficients fitted
# end-to-end against the reference (see fit in dev notes).
PC0 = 0.11695361
PC1 = 0.9984974
PC2 = 1.00543106


def _register_poly_phi():
    """Register the fused single-pass phi op with the custom-DVE registry."""
    import concourse.dve_ops as dve_ops
    from concourse.dve_ops import DveOp, OPS, _SUB_OPCODE_FOR_NAME, _CUSTOM_DVE_ROW_BASE
    from concourse.dve_spec import Spec, Src0, C0, C1, C2, relu, sq, minn, lower, _has_src1
    from concourse.dve_uop import DveOpSpec

    name = "POLY_PHI_AN8"
    for op in OPS:
        if op.name == name:
            return op

    def ref_poly_phi(in0, in1, c0, c1, c2):
        u = (c1 + c0 * np.asarray(in0, dtype=np.float32)).astype(np.float32)
        u = (u * u).astype(np.float32)
        u = (u * u).astype(np.float32)
        u = (u * u).astype(np.float32)
        return np.minimum(u, c2) + np.maximum(in0, 0.0).astype(np.float32)

    spec = Spec(
        body=minn(sq(sq(sq(Src0 * C0 + C1))), C2) + relu(Src0),
        reference=ref_poly_phi,
    )
    opcode = _CUSTOM_DVE_ROW_BASE + len(OPS)
    shas = {}
    for ver in ("v3", "v4"):
        try:
            s = DveOpSpec(name=name, opcode=opcode, uops=lower(spec, ver=ver),
                          rd1_en=_has_src1(spec))
            shas[ver] = s.sha(ver)
        except Exception:
            pass
    op = DveOp(name, spec, subdim=False, uops_sha=shas)
    OPS.append(op)
    _SUB_OPCODE_FOR_NAME[name] = opcode
    dve_ops.CUSTOM_DVE_SPECS[name] = spec
    return op


POLY_PHI = _register_poly_phi()


def flat2(ap):
    return ap.rearrange("p a b -> p (a b)")


def build_kernel(repeat: int = 1, xp_bufs=2, pqvp_bufs=2, phikp_bufs=4, mm_bufs=3,
                 outp_bufs=4, tmps_bufs=6, n_schemeg=13, out_act_mod=99):
    """Build the per-core Bass program.

    n_schemeg: how many of the 16 phi_k spans per core use ACT Exp/Relu +
    GPSIMD bf16 combine instead of the fused poly DVE op (load balance knob).
    out_act_mod: out-copy engine split; span uses DVE when
    (idx % out_act_mod) == out_act_mod-1, else ACT.
    """
    nc = bacc.Bacc("TRN2", target_bir_lowering=False, debug=False)

    x_d = nc.dram_tensor("x", [NB, 128, 2, HW], F8, kind="ExternalInput")
    wqv_d = nc.dram_tensor("wqv", [128, 2, 512], F8, kind="ExternalInput")
    wk_d = nc.dram_tensor("wk", [128, 2, 256], F8, kind="ExternalInput")
    wo_d = nc.dram_tensor("wo", [128, 2, 256], F32R, kind="ExternalInput")
    out_d = nc.dram_tensor("out", [NB, 2, 128, HW], BF, kind="ExternalOutput")

    with tile.TileContext(nc) as tc, ExitStack() as ctx:
        singles = ctx.enter_context(tc.tile_pool(name="singles", bufs=1))
        xp = ctx.enter_context(tc.tile_pool(name="xp", bufs=xp_bufs))
        pqvp = ctx.enter_context(tc.tile_pool(name="pqvp", bufs=pqvp_bufs))
        phikp = ctx.enter_context(tc.tile_pool(name="phikp", bufs=phikp_bufs))
        tmps = ctx.enter_context(tc.tile_pool(name="tmps", bufs=tmps_bufs))
        smalls = ctx.enter_context(tc.tile_pool(name="smalls", bufs=4))
        outp = ctx.enter_context(tc.tile_pool(name="outp", bufs=outp_bufs))
        psmm = ctx.enter_context(tc.tile_pool(name="psmm", bufs=mm_bufs, space="PSUM"))
        psacc = ctx.enter_context(tc.tile_pool(name="psacc", bufs=1, space="PSUM"))

        # ---- weights (loaded once, replicated) ----
        wqv_sb = singles.tile([128, 2, 512], F8, tag="wqv")
        nc.sync.dma_start(out=wqv_sb[:], in_=wqv_d.ap())
        wk_sb = singles.tile([128, 2, 256], F8, tag="wk")
        nc.sync.dma_start(out=wk_sb[:], in_=wk_d.ap())
        wo_sb = singles.tile([128, 2, 256], F32R, tag="wo")
        nc.sync.dma_start(out=wo_sb[:], in_=wo_d.ap())

        state = {"out": 0, "aspan": 0}

        def poly_phi(psum_ap, dst_ap):
            nc.vector._custom_dve(POLY_PHI, out=dst_ap, in0=psum_ap,
                                  s0=PC0, s1=PC1, imm2=PC2)

        def phi_schemeg(psum_ap, dst_ap):
            """ACT-heavy phi: e=Exp(x); r=Relu(x); t=min(e,1) (DVE 4x);
            dst=t+r (GPSIMD, the otherwise-idle engine)."""
            e = tmps.tile([128, 1024], BF, tag="e")
            nc.scalar.activation(e[:], psum_ap, AF.Exp)
            r = tmps.tile([128, 1024], BF, tag="r")
            nc.scalar.activation(r[:], psum_ap, AF.Relu)
            t = tmps.tile([128, 1024], BF, tag="t")
            nc.vector.tensor_scalar_min(t[:], e[:], 1.0)
            nc.gpsimd.tensor_tensor(dst_ap, t[:], r[:], OP.add)

        def load_x(X, b):
            xblocks = [(0, 512), (512, 1536), (2048, 2048)]
            for (c0, cw) in xblocks:
                cs = slice(c0, c0 + cw)
                nc.sync.dma_start(out=X[:, :, cs], in_=x_d.ap()[b, :, :, cs])

        def b_span(X, pqv, i, qv_ps):
            # one stage-B span: phi(q^T), phi(v^T) chunk pair -> pqv[:, i],
            # then immediately fold chunk i into the qv accumulation (stage C)
            ps = psmm.tile([128, 2, 512], FP, tag="mm")
            for j in range(2):
                nk = i * 2 + j
                nc.tensor.matmul(
                    ps[:, j, :],
                    X[:, :, nk * 128:(nk + 1) * 128],
                    wqv_sb[:],
                    start=True, stop=True,
                    perf_mode=DR,
                )
            poly_phi(flat2(ps[:]), flat2(pqv[:, i]))
            for qh in range(2):
                nc.tensor.matmul(
                    qv_ps[:, qh, 0:256],
                    pqv[:, i, :, qh * 128:qh * 128 + 128],
                    pqv[:, i, :, 256:512],
                    start=(i == 0), stop=(i == 15),
                    perf_mode=DR,
                )

        def a_span(X, phik, si):
            # one stage-A span: phi_k block si (m = si//4, i = si%4)
            m, i = si // 4, si % 4
            ps = psmm.tile([128, 2, 512], FP, tag="mm")
            for j in range(2):
                n0 = (i * 2 + j) * 512
                nc.tensor.matmul(
                    ps[:, j, :],
                    wk_sb[:, :, m * 128:(m + 1) * 128],
                    X[:, :, n0:n0 + 512],
                    start=True, stop=True,
                    perf_mode=DR,
                )
            dst = phik[m][:, i * 1024:(i + 1) * 1024]
            if state["aspan"] % 16 < n_schemeg:
                phi_schemeg(flat2(ps[:]), dst)
            else:
                poly_phi(flat2(ps[:]), dst)
            state["aspan"] += 1

        def stage_C(qv_ps):
            # qv accumulation happened inline in b_span; here just the
            # qv copy + W2^T[d, o] = sum_c qv[c, d] WoT[c, o] chain
            # latency-critical copies run on DVE: at window end the ACT queue
            # has a multi-op backlog while DVE has just drained its polys
            qv_sb = smalls.tile([128, 2, 256], F32R, tag="qv_sb")
            nc.vector.tensor_scalar_add(qv_sb[:], qv_ps[:, :, 0:256], 0.0)

            w2_ps = psmm.tile([128, 2, 256], FP, tag="mm", name="w2ps")
            for dh in range(2):
                for cc in range(2):
                    nc.tensor.matmul(
                        w2_ps[:, dh, :],
                        qv_sb[:, cc, dh * 128:(dh + 1) * 128],
                        wo_sb[:, cc, :],
                        start=(cc == 0), stop=(cc == 1),
                    )
            w2_sb = smalls.tile([128, 2, 256], BF, tag="w2_sb")
            nc.vector.tensor_scalar_add(flat2(w2_sb[:]), flat2(w2_ps[:]), 0.0)
            return w2_sb

        def d_span(prev, si, ostage):
            b, w2_sb, phik = prev
            m, i = si // 4, si % 4
            ps = psmm.tile([128, 2, 512], FP, tag="mm")
            for j in range(2):
                n0 = (i * 2 + j) * 512
                for dd in range(2):
                    nc.tensor.matmul(
                        ps[:, j, :],
                        w2_sb[:, dd, m * 128:(m + 1) * 128],
                        phik[dd][:, n0:n0 + 512],
                        start=(dd == 0), stop=(dd == 1),
                    )
            o_sb = ostage[m]
            dst = o_sb[:, i * 1024:(i + 1) * 1024]
            if state["out"] % out_act_mod == out_act_mod - 1:
                nc.vector.tensor_scalar_add(dst, flat2(ps[:]), 0.0)
            else:
                nc.scalar.activation(dst, flat2(ps[:]), AF.Copy)
            state["out"] += 1
            if i % 2 == 1:
                # consolidated DMA per 2048-column half
                n0 = (i - 1) * 1024
                nc.sync.dma_start(
                    out=out_d.ap()[b, m, :, n0:n0 + 2048],
                    in_=o_sb[:, n0:n0 + 2048],
                )

        # ---- persistent ping-pong slots (explicit 2-deep pipeline) ----
        X_slots = [singles.tile([128, 2, HW], F8, tag=f"x{p}", name=f"xs{p}")
                   for p in range(2)]
        pqv_slots = [singles.tile([128, 16, 2, 512], F8, tag=f"pqv{p}", name=f"pqvs{p}")
                     for p in range(2)]
        phik_slots = [[singles.tile([128, HW], BF, tag=f"phik{p}_{m}", name=f"pks{p}_{m}")
                       for m in range(2)] for p in range(2)]
        qv_ps = psacc.tile([128, 2, 512], FP, tag="acc", name="qvps")

        def unit(t, prev):
            """One batch-unit: B+A window for batch t%2 with the previous
            unit's D-spans woven in, then the C/C2 chain."""
            bt = t % 2
            X, pqv, phik = X_slots[bt], pqv_slots[bt], phik_slots[bt]
            load_x(X, bt)
            ostage = None
            if prev is not None:
                ostage = [outp.tile([128, HW], BF, tag="osb", name=f"osb{t%4}_{m}")
                          for m in range(2)]
            # A-spans (depend only on X, never stall) at the window edges;
            # D-spans of the previous unit in the middle (i=4..11): late
            # enough that w2 is ready (no PE head-of-line blocking), early
            # enough that PE's heavier D-matmul load spreads across the window
            na = 0
            for i in range(16):
                b_span(X, pqv, i, qv_ps)
                if 4 <= i < 12 and prev is not None:
                    d_span(prev, i - 4, ostage)
                else:
                    if na < 8:
                        a_span(X, phik, na)
                        na += 1
            while na < 8:
                a_span(X, phik, na)
                na += 1
            w2 = stage_C(qv_ps)
            return (bt, w2, phik)

        def drain_d(prev):
            ostage = [outp.tile([128, HW], BF, tag="osb", name=f"osbd_{m}")
                      for m in range(2)]
            for si in range(8):
                d_span(prev, si, ostage)

        def emit(n_bodies, _iv=None):
            state["out"] = 0
            state["aspan"] = 0
            prev = None
            for t in range(2 * n_bodies):
                prev = unit(t, prev)
            drain_d(prev)

        if repeat <= 4:
            emit(repeat)
        else:
            # unroll bodies inside the hardware loop: plain For_i has an
            # all-engine barrier per iteration (a full pipeline drain);
            # unrolling amortizes it, and the per-iteration D-drain tail
            # doubles as the loop-carried-dependence safety net
            unroll = 1
            for u in (8, 4, 2):
                if repeat % u == 0:
                    unroll = u
                    break
            with tc.For_i(0, repeat // unroll, 1) as iv:
                emit(unroll, iv)

    nc.compile()
    return nc


_nc_cache = {}


def _get_nc(repeat: int = 1):
    if repeat not in _nc_cache:
        _nc_cache[repeat] = build_kernel(repeat)
    return _nc_cache[repeat]


def make_in_maps(x, Wq, Wk, Wv, Wo, bo):
    import ml_dtypes

    f8 = np.dtype(ml_dtypes.float8_e4m3)
    # x: [B, C, H, W] -> per-core [NB, 128, 2, HW] fp8 (p-major, cc interleave)
    x8 = np.asarray(x, dtype=np.float32).reshape(B, 2, 128, HW).transpose(0, 2, 1, 3)
    x8 = np.ascontiguousarray(x8).astype(f8)
    # wqv[p, cc, 0:256] = Wq.T[cc*128+p, :], [...,256:512] = Wv.T
    wqt = np.asarray(Wq, dtype=np.float32).T.reshape(2, 128, 256)
    wvt = np.asarray(Wv, dtype=np.float32).T.reshape(2, 128, 256)
    wqv = np.concatenate([wqt, wvt], axis=2).transpose(1, 0, 2)
    wqv = np.ascontiguousarray(wqv).astype(f8)
    wkt = np.asarray(Wk, dtype=np.float32).T.reshape(2, 128, 256).transpose(1, 0, 2)
    wk8 = np.ascontiguousarray(wkt).astype(f8)
    wot = np.asarray(Wo, dtype=np.float32).T.reshape(2, 128, 256).transpose(1, 0, 2)
    wo32 = np.ascontiguousarray(wot)
    return [
        {"x": x8[i * NB:(i + 1) * NB], "wqv": wqv, "wk": wk8, "wo": wo32}
        for i in range(NCORES)
    ]


def kernel(x, Wq, Wk, Wv, Wo, bo):
    nc = _get_nc(repeat=1)
    in_maps = make_in_maps(x, Wq, Wk, Wv, Wo, bo)
    res = bass_utils.run_bass_kernel_spmd(nc, in_maps, core_ids=list(range(NCORES)))
    out = np.concatenate([res.results[i]["out"] for i in range(NCORES)], axis=0)
    out = out.astype(np.float32).reshape(B, C, H, W)
    out += np.asarray(bo, dtype=np.float32)[None, :, None, None]
    return np.ascontiguousarray(out)


# revision 37
# speedup vs baseline: 1.0571x; 1.0571x over previous
"""Trainium2 Bass kernel for ConvolutionalAttention2D (linear attention with 1x1 convs).

Reference computation (per batch b):
    q = Wq x ; k = Wk x ; v = Wv x          (1x1 convs == channel matmuls)
    phi(t) = elu(t) + 1
    qv = phi(q) @ phi(v)^T                  ([C, C] context matrix, contract over pixels)
    out = Wo (qv @ phi(k)) + bo

Kernel strategy (8 NeuronCores, data-parallel over batch B=16 -> 2 batches/core):
  - Projections (q,v transposed layout + k natural layout) and the qv
    contraction run as fp8(e4m3) DoubleRow matmuls: contraction dim 256 =
    2 k-tiles processed 2 rows/cycle -> half the PE time of bf16.
  - phi(t) = elu(t)+1 is approximated by a single fused custom DVE op:
        phi(t) ~= min((C1 + C0*t)^8, C2) + relu(t)
    with coefficients fitted end-to-end (rel err ~2e-3 incl. fp8). One
    PSUM pass per phi span instead of ACT-exp + DVE fixup.
  - Most phi_k spans use ACT (Exp+Relu) + DVE min + GPSIMD add ("scheme G")
    to balance load across all four engines.
  - Stage D (out = (Wo qv) @ phi_k) stays bf16 (fp8 W2 overflows/too coarse).
  - bo is added on the host (it's a [C] broadcast; free there).
  - Output written bf16, upcast on host.
  - Execution is a flat pipeline of batch-units: each unit runs one batch's
    B/A window with the previous unit's stage-D spans woven into the middle
    (head-of-line-safe), qv accumulation inline; For_i iterations unroll 8
    bodies to amortize the loop's all-engine barrier.
"""

from contextlib import ExitStack

import numpy as np

import concourse.bacc as bacc
import concourse.tile as tile
from concourse import mybir
from concourse import bass_utils

B, C, H, W = 16, 256, 64, 64
HW = H * W
NCORES = 8
NB = B // NCORES  # batches per core

FP = mybir.dt.float32
BF = mybir.dt.bfloat16
F32R = mybir.dt.float32r
F8 = mybir.dt.float8e4
AF = mybir.ActivationFunctionType
OP = mybir.AluOpType
DR = mybir.MatmulPerfMode.DoubleRow

# phi(t) ~= min((PC1 + PC0*t)^8, PC2) + relu(t), coefficients fitted
# end-to-end against the reference (see fit in dev notes).
PC0 = 0.11695361
PC1 = 0.9984974
PC2 = 1.00543106


def _register_poly_phi():
    """Register the fused single-pass phi op with the custom-DVE registry."""
    import concourse.dve_ops as dve_ops
    from concourse.dve_ops import DveOp, OPS, _SUB_OPCODE_FOR_NAME, _CUSTOM_DVE_ROW_BASE
    from concourse.dve_spec import Spec, Src0, C0, C1, C2, relu, sq, minn, lower, _has_src1
    from concourse.dve_uop import DveOpSpec

    name = "POLY_PHI_AN8"
    for op in OPS:
        if op.name == name:
            return op

    def ref_poly_phi(in0, in1, c0, c1, c2):
        u = (c1 + c0 * np.asarray(in0, dtype=np.float32)).astype(np.float32)
        u = (u * u).astype(np.float32)
        u = (u * u).astype(np.float32)
        u = (u * u).astype(np.float32)
        return np.minimum(u, c2) + np.maximum(in0, 0.0).astype(np.float32)

    spec = Spec(
        body=minn(sq(sq(sq(Src0 * C0 + C1))), C2) + relu(Src0),
        reference=ref_poly_phi,
    )
    opcode = _CUSTOM_DVE_ROW_BASE + len(OPS)
    shas = {}
    for ver in ("v3", "v4"):
        try:
            s = DveOpSpec(name=name, opcode=opcode, uops=lower(spec, ver=ver),
                          rd1_en=_has_src1(spec))
            shas[ver] = s.sha(ver)
        except Exception:
            pass
    op = DveOp(name, spec, subdim=False, uops_sha=shas)
    OPS.append(op)
    _SUB_OPCODE_FOR_NAME[name] = opcode
    dve_ops.CUSTOM_DVE_SPECS[name] = spec
    return op


POLY_PHI = _register_poly_phi()


def flat2(ap):
    return ap.rearrange("p a b -> p (a b)")


def build_kernel(repeat: int = 1, xp_bufs=2, pqvp_bufs=2, phikp_bufs=4, mm_bufs=3,
                 outp_bufs=4, tmps_bufs=6, n_schemeg=13, out_act_mod=99):
    """Build the per-core Bass program.

    n_schemeg: how many of the 16 phi_k spans per core use ACT Exp/Relu +
    GPSIMD bf16 combine instead of the fused poly DVE op (load balance knob).
    out_act_mod: out-copy engine split; span uses DVE when
    (idx % out_act_mod) == out_act_mod-1, else ACT.
    """
    nc = bacc.Bacc("TRN2", target_bir_lowering=False, debug=False)

    x_d = nc.dram_tensor("x", [NB, 128, 2, HW], F8, kind="ExternalInput")
    wqv_d = nc.dram_tensor("wqv", [128, 2, 512], F8, kind="ExternalInput")
    wk_d = nc.dram_tensor("wk", [128, 2, 256], F8, kind="ExternalInput")
    wo_d = nc.dram_tensor("wo", [128, 2, 256], F32R, kind="ExternalInput")
    out_d = nc.dram_tensor("out", [NB, 2, 128, HW], BF, kind="ExternalOutput")

    with tile.TileContext(nc) as tc, ExitStack() as ctx:
        singles = ctx.enter_context(tc.tile_pool(name="singles", bufs=1))
        xp = ctx.enter_context(tc.tile_pool(name="xp", bufs=xp_bufs))
        pqvp = ctx.enter_context(tc.tile_pool(name="pqvp", bufs=pqvp_bufs))
        phikp = ctx.enter_context(tc.tile_pool(name="phikp", bufs=phikp_bufs))
        tmps = ctx.enter_context(tc.tile_pool(name="tmps", bufs=tmps_bufs))
        smalls = ctx.enter_context(tc.tile_pool(name="smalls", bufs=4))
        outp = ctx.enter_context(tc.tile_pool(name="outp", bufs=outp_bufs))
        psmm = ctx.enter_context(tc.tile_pool(name="psmm", bufs=mm_bufs, space="PSUM"))
        psacc = ctx.enter_context(tc.tile_pool(name="psacc", bufs=1, space="PSUM"))

        # ---- weights (loaded once, replicated) ----
        wqv_sb = singles.tile([128, 2, 512], F8, tag="wqv")
        nc.sync.dma_start(out=wqv_sb[:], in_=wqv_d.ap())
        wk_sb = singles.tile([128, 2, 256], F8, tag="wk")
        nc.sync.dma_start(out=wk_sb[:], in_=wk_d.ap())
        wo_sb = singles.tile([128, 2, 256], F32R, tag="wo")
        nc.sync.dma_start(out=wo_sb[:], in_=wo_d.ap())

        state = {"out": 0, "aspan": 0}

        def poly_phi(psum_ap, dst_ap):
            nc.vector._custom_dve(POLY_PHI, out=dst_ap, in0=psum_ap,
                                  s0=PC0, s1=PC1, imm2=PC2)

        def phi_schemeg(psum_ap, dst_ap):
            """ACT-heavy phi: e=Exp(x); r=Relu(x); t=min(e,1) (DVE 4x);
            dst=t+r (GPSIMD, the otherwise-idle engine)."""
            e = tmps.tile([128, 1024], BF, tag="e")
            nc.scalar.activation(e[:], psum_ap, AF.Exp)
            r = tmps.tile([128, 1024], BF, tag="r")
            nc.scalar.activation(r[:], psum_ap, AF.Relu)
            t = tmps.tile([128, 1024], BF, tag="t")
            nc.vector.tensor_scalar_min(t[:], e[:], 1.0)
            nc.gpsimd.tensor_tensor(dst_ap, t[:], r[:], OP.add)

        def load_x(X, b):
            xblocks = [(0, 512), (512, 1536), (2048, 2048)]
            for (c0, cw) in xblocks:
                cs = slice(c0, c0 + cw)
                nc.sync.dma_start(out=X[:, :, cs], in_=x_d.ap()[b, :, :, cs])

        def qv_accum(pqv, i, qv_ps):
            # fold pqv chunk i into the qv accumulation (stage C)
            for qh in range(2):
                nc.tensor.matmul(
                    qv_ps[:, qh, 0:256],
                    pqv[:, i, :, qh * 128:qh * 128 + 128],
                    pqv[:, i, :, 256:512],
                    start=(i == 0), stop=(i == 15),
                    perf_mode=DR,
                )

        def b_span(X, pqv, i, qv_ps):
            # one stage-B span: phi(q^T), phi(v^T) chunk pair -> pqv[:, i].
            # The qv accumulation for chunk i-1 is emitted here (one span
            # later) so its poly has surely completed when PE reaches it --
            # no head-of-line blocking in the PE wait queue.
            ps = psmm.tile([128, 2, 512], FP, tag="mm")
            for j in range(2):
                nk = i * 2 + j
                nc.tensor.matmul(
                    ps[:, j, :],
                    X[:, :, nk * 128:(nk + 1) * 128],
                    wqv_sb[:],
                    start=True, stop=True,
                    perf_mode=DR,
                )
            poly_phi(flat2(ps[:]), flat2(pqv[:, i]))
            if i > 0:
                qv_accum(pqv, i - 1, qv_ps)

        def a_span(X, phik, si):
            # one stage-A span: phi_k block si (m = si//4, i = si%4)
            m, i = si // 4, si % 4
            ps = psmm.tile([128, 2, 512], FP, tag="mm")
            for j in range(2):
                n0 = (i * 2 + j) * 512
                nc.tensor.matmul(
                    ps[:, j, :],
                    wk_sb[:, :, m * 128:(m + 1) * 128],
                    X[:, :, n0:n0 + 512],
                    start=True, stop=True,
                    perf_mode=DR,
                )
            dst = phik[m][:, i * 1024:(i + 1) * 1024]
            if state["aspan"] % 16 < n_schemeg:
                phi_schemeg(flat2(ps[:]), dst)
            else:
                poly_phi(flat2(ps[:]), dst)
            state["aspan"] += 1

        def stage_C(qv_ps):
            # qv accumulation happened inline in b_span; here just the
            # qv copy + W2^T[d, o] = sum_c qv[c, d] WoT[c, o] chain
            # latency-critical copies run on DVE: at window end the ACT queue
            # has a multi-op backlog while DVE has just drained its polys
            qv_sb = smalls.tile([128, 2, 256], F32R, tag="qv_sb")
            nc.vector.tensor_scalar_add(qv_sb[:], qv_ps[:, :, 0:256], 0.0)

            w2_ps = psmm.tile([128, 2, 256], FP, tag="mm", name="w2ps")
            for dh in range(2):
                for cc in range(2):
                    nc.tensor.matmul(
                        w2_ps[:, dh, :],
                        qv_sb[:, cc, dh * 128:(dh + 1) * 128],
                        wo_sb[:, cc, :],
                        start=(cc == 0), stop=(cc == 1),
                    )
            w2_sb = smalls.tile([128, 2, 256], BF, tag="w2_sb")
            nc.vector.tensor_scalar_add(flat2(w2_sb[:]), flat2(w2_ps[:]), 0.0)
            return w2_sb

        def d_span(prev, si, ostage):
            b, w2_sb, phik = prev
            m, i = si // 4, si % 4
            ps = psmm.tile([128, 2, 512], FP, tag="mm")
            for j in range(2):
                n0 = (i * 2 + j) * 512
                for dd in range(2):
                    nc.tensor.matmul(
                        ps[:, j, :],
                        w2_sb[:, dd, m * 128:(m + 1) * 128],
                        phik[dd][:, n0:n0 + 512],
                        start=(dd == 0), stop=(dd == 1),
                    )
            o_sb = ostage[m]
            dst = o_sb[:, i * 1024:(i + 1) * 1024]
            if state["out"] % out_act_mod == out_act_mod - 1:
                nc.vector.tensor_scalar_add(dst, flat2(ps[:]), 0.0)
            else:
                nc.scalar.activation(dst, flat2(ps[:]), AF.Copy)
            state["out"] += 1
            if i % 2 == 1:
                # consolidated DMA per 2048-column half
                n0 = (i - 1) * 1024
                nc.sync.dma_start(
                    out=out_d.ap()[b, m, :, n0:n0 + 2048],
                    in_=o_sb[:, n0:n0 + 2048],
                )

        # ---- persistent ping-pong slots (explicit 2-deep pipeline) ----
        X_slots = [singles.tile([128, 2, HW], F8, tag=f"x{p}", name=f"xs{p}")
                   for p in range(2)]
        pqv_slots = [singles.tile([128, 16, 2, 512], F8, tag=f"pqv{p}", name=f"pqvs{p}")
                     for p in range(2)]
        phik_slots = [[singles.tile([128, HW], BF, tag=f"phik{p}_{m}", name=f"pks{p}_{m}")
                       for m in range(2)] for p in range(2)]
        qv_ps = psacc.tile([128, 2, 512], FP, tag="acc", name="qvps")

        def unit(t, prev):
            """One batch-unit: B+A window for batch t%2 with the previous
            unit's D-spans woven in, then the C/C2 chain."""
            bt = t % 2
            X, pqv, phik = X_slots[bt], pqv_slots[bt], phik_slots[bt]
            load_x(X, bt)
            ostage = None
            if prev is not None:
                ostage = [outp.tile([128, HW], BF, tag="osb", name=f"osb{t%4}_{m}")
                          for m in range(2)]
            # A-spans (depend only on X, never stall) at the window edges;
            # D-spans of the previous unit in the middle (i=4..11): late
            # enough that w2 is ready (no PE head-of-line blocking), early
            # enough that PE's heavier D-matmul load spreads across the window
            na = 0
            for i in range(16):
                b_span(X, pqv, i, qv_ps)
                if 4 <= i < 12 and prev is not None:
                    d_span(prev, i - 4, ostage)
                else:
                    if na < 8:
                        a_span(X, phik, na)
                        na += 1
            while na < 8:
                a_span(X, phik, na)
                na += 1
            qv_accum(pqv, 15, qv_ps)
            w2 = stage_C(qv_ps)
            return (bt, w2, phik)

        def drain_d(prev):
            ostage = [outp.tile([128, HW], BF, tag="osb", name=f"osbd_{m}")
                      for m in range(2)]
            for si in range(8):
                d_span(prev, si, ostage)

        def emit(n_bodies, _iv=None):
            state["out"] = 0
            state["aspan"] = 0
            prev = None
            for t in range(2 * n_bodies):
                prev = unit(t, prev)
            drain_d(prev)

        if repeat <= 4:
            emit(repeat)
        else:
            # unroll bodies inside the hardware loop: plain For_i has an
            # all-engine barrier per iteration (a full pipeline drain);
            # unrolling amortizes it, and the per-iteration D-drain tail
            # doubles as the loop-carried-dependence safety net
            unroll = 1
            for u in (8, 4, 2):
                if repeat % u == 0:
                    unroll = u
                    break
            with tc.For_i(0, repeat // unroll, 1) as iv:
                emit(unroll, iv)

    nc.compile()
    return nc


_nc_cache = {}


def _get_nc(repeat: int = 1):
    if repeat not in _nc_cache:
        _nc_cache[repeat] = build_kernel(repeat)
    return _nc_cache[repeat]


def make_in_maps(x, Wq, Wk, Wv, Wo, bo):
    import ml_dtypes

    f8 = np.dtype(ml_dtypes.float8_e4m3)
    # x: [B, C, H, W] -> per-core [NB, 128, 2, HW] fp8 (p-major, cc interleave)
    x8 = np.asarray(x, dtype=np.float32).reshape(B, 2, 128, HW).transpose(0, 2, 1, 3)
    x8 = np.ascontiguousarray(x8).astype(f8)
    # wqv[p, cc, 0:256] = Wq.T[cc*128+p, :], [...,256:512] = Wv.T
    wqt = np.asarray(Wq, dtype=np.float32).T.reshape(2, 128, 256)
    wvt = np.asarray(Wv, dtype=np.float32).T.reshape(2, 128, 256)
    wqv = np.concatenate([wqt, wvt], axis=2).transpose(1, 0, 2)
    wqv = np.ascontiguousarray(wqv).astype(f8)
    wkt = np.asarray(Wk, dtype=np.float32).T.reshape(2, 128, 256).transpose(1, 0, 2)
    wk8 = np.ascontiguousarray(wkt).astype(f8)
    wot = np.asarray(Wo, dtype=np.float32).T.reshape(2, 128, 256).transpose(1, 0, 2)
    wo32 = np.ascontiguousarray(wot)
    return [
        {"x": x8[i * NB:(i + 1) * NB], "wqv": wqv, "wk": wk8, "wo": wo32}
        for i in range(NCORES)
    ]


def kernel(x, Wq, Wk, Wv, Wo, bo):
    nc = _get_nc(repeat=1)
    in_maps = make_in_maps(x, Wq, Wk, Wv, Wo, bo)
    res = bass_utils.run_bass_kernel_spmd(nc, in_maps, core_ids=list(range(NCORES)))
    out = np.concatenate([res.results[i]["out"] for i in range(NCORES)], axis=0)
    out = out.astype(np.float32).reshape(B, C, H, W)
    out += np.asarray(bo, dtype=np.float32)[None, :, None, None]
    return np.ascontiguousarray(out)


# revision 38
# speedup vs baseline: 1.2891x; 1.2196x over previous
"""Trainium2 Bass kernel for ConvolutionalAttention2D (linear attention with 1x1 convs).

Reference computation (per batch b):
    q = Wq x ; k = Wk x ; v = Wv x          (1x1 convs == channel matmuls)
    phi(t) = elu(t) + 1
    qv = phi(q) @ phi(v)^T                  ([C, C] context matrix, contract over pixels)
    out = Wo (qv @ phi(k)) + bo

Kernel strategy (8 NeuronCores, data-parallel over batch B=16 -> 2 batches/core):
  - Projections (q,v transposed layout + k natural layout) and the qv
    contraction run as fp8(e4m3) DoubleRow matmuls: contraction dim 256 =
    2 k-tiles processed 2 rows/cycle -> half the PE time of bf16.
  - phi(t) = elu(t)+1 is approximated by a single fused custom DVE op:
        phi(t) ~= min((C1 + C0*t)^8, C2) + relu(t)
    with coefficients fitted end-to-end (rel err ~2e-3 incl. fp8). One
    PSUM pass per phi span instead of ACT-exp + DVE fixup.
  - Most phi_k spans use ACT (Exp+Relu) + DVE min + GPSIMD add ("scheme G")
    to balance load across all four engines.
  - Stage D (out = (Wo qv) @ phi_k) stays bf16 (fp8 W2 overflows/too coarse).
  - bo is added on the host (it's a [C] broadcast; free there).
  - Output written bf16, upcast on host.
  - Execution is a flat pipeline of batch-units: each unit runs one batch's
    B/A window with the previous unit's stage-D spans woven into the middle
    (head-of-line-safe), qv accumulation inline; For_i iterations unroll 8
    bodies to amortize the loop's all-engine barrier.
"""

from contextlib import ExitStack

import numpy as np

import concourse.bacc as bacc
import concourse.tile as tile
from concourse import mybir
from concourse import bass_utils

B, C, H, W = 16, 256, 64, 64
HW = H * W
NCORES = 8
NB = B // NCORES  # batches per core

FP = mybir.dt.float32
BF = mybir.dt.bfloat16
F32R = mybir.dt.float32r
F8 = mybir.dt.float8e4
AF = mybir.ActivationFunctionType
OP = mybir.AluOpType
DR = mybir.MatmulPerfMode.DoubleRow

# phi(t) ~= min((PC1 + PC0*t)^8, PC2) + relu(t), coefficients fitted
# end-to-end against the reference (see fit in dev notes).
PC0 = 0.11695361
PC1 = 0.9984974
PC2 = 1.00543106


def _register_poly_phi():
    """Register the fused single-pass phi op with the custom-DVE registry."""
    import concourse.dve_ops as dve_ops
    from concourse.dve_ops import DveOp, OPS, _SUB_OPCODE_FOR_NAME, _CUSTOM_DVE_ROW_BASE
    from concourse.dve_spec import Spec, Src0, C0, C1, C2, relu, sq, minn, lower, _has_src1
    from concourse.dve_uop import DveOpSpec

    name = "POLY_PHI_AN8"
    for op in OPS:
        if op.name == name:
            return op

    def ref_poly_phi(in0, in1, c0, c1, c2):
        u = (c1 + c0 * np.asarray(in0, dtype=np.float32)).astype(np.float32)
        u = (u * u).astype(np.float32)
        u = (u * u).astype(np.float32)
        u = (u * u).astype(np.float32)
        return np.minimum(u, c2) + np.maximum(in0, 0.0).astype(np.float32)

    spec = Spec(
        body=minn(sq(sq(sq(Src0 * C0 + C1))), C2) + relu(Src0),
        reference=ref_poly_phi,
    )
    opcode = _CUSTOM_DVE_ROW_BASE + len(OPS)
    shas = {}
    for ver in ("v3", "v4"):
        try:
            s = DveOpSpec(name=name, opcode=opcode, uops=lower(spec, ver=ver),
                          rd1_en=_has_src1(spec))
            shas[ver] = s.sha(ver)
        except Exception:
            pass
    op = DveOp(name, spec, subdim=False, uops_sha=shas)
    OPS.append(op)
    _SUB_OPCODE_FOR_NAME[name] = opcode
    dve_ops.CUSTOM_DVE_SPECS[name] = spec
    return op


POLY_PHI = _register_poly_phi()


def flat2(ap):
    return ap.rearrange("p a b -> p (a b)")


def build_kernel(repeat: int = 1, xp_bufs=2, pqvp_bufs=2, phikp_bufs=4, mm_bufs=3,
                 outp_bufs=4, tmps_bufs=6, n_schemeg=13, out_act_mod=99):
    """Build the per-core Bass program.

    n_schemeg: how many of the 16 phi_k spans per core use ACT Exp/Relu +
    GPSIMD bf16 combine instead of the fused poly DVE op (load balance knob).
    out_act_mod: out-copy engine split; span uses DVE when
    (idx % out_act_mod) == out_act_mod-1, else ACT.
    """
    nc = bacc.Bacc("TRN2", target_bir_lowering=False, debug=False)

    x_d = nc.dram_tensor("x", [NB, 128, 2, HW], F8, kind="ExternalInput")
    wqv_d = nc.dram_tensor("wqv", [128, 2, 512], F8, kind="ExternalInput")
    wk_d = nc.dram_tensor("wk", [128, 2, 256], F8, kind="ExternalInput")
    wo_d = nc.dram_tensor("wo", [128, 2, 256], F32R, kind="ExternalInput")
    out_d = nc.dram_tensor("out", [NB, 2, 128, HW], BF, kind="ExternalOutput")

    with tile.TileContext(nc) as tc, ExitStack() as ctx:
        singles = ctx.enter_context(tc.tile_pool(name="singles", bufs=1))
        xp = ctx.enter_context(tc.tile_pool(name="xp", bufs=xp_bufs))
        pqvp = ctx.enter_context(tc.tile_pool(name="pqvp", bufs=pqvp_bufs))
        phikp = ctx.enter_context(tc.tile_pool(name="phikp", bufs=phikp_bufs))
        tmps = ctx.enter_context(tc.tile_pool(name="tmps", bufs=tmps_bufs))
        smalls = ctx.enter_context(tc.tile_pool(name="smalls", bufs=4))
        outp = ctx.enter_context(tc.tile_pool(name="outp", bufs=outp_bufs))
        psmm = ctx.enter_context(tc.tile_pool(name="psmm", bufs=mm_bufs, space="PSUM"))
        psacc = ctx.enter_context(tc.tile_pool(name="psacc", bufs=1, space="PSUM"))

        # ---- weights (loaded once, replicated) ----
        wqv_sb = singles.tile([128, 2, 512], F8, tag="wqv")
        nc.sync.dma_start(out=wqv_sb[:], in_=wqv_d.ap())
        wk_sb = singles.tile([128, 2, 256], F8, tag="wk")
        nc.sync.dma_start(out=wk_sb[:], in_=wk_d.ap())
        wo_sb = singles.tile([128, 2, 256], F32R, tag="wo")
        nc.sync.dma_start(out=wo_sb[:], in_=wo_d.ap())

        state = {"out": 0, "aspan": 0}

        def poly_phi(psum_ap, dst_ap):
            nc.vector._custom_dve(POLY_PHI, out=dst_ap, in0=psum_ap,
                                  s0=PC0, s1=PC1, imm2=PC2)

        def phi_schemeg(psum_ap, dst_ap):
            """ACT-heavy phi: e=Exp(x); r=Relu(x); t=min(e,1) (DVE 4x);
            dst=t+r (GPSIMD, the otherwise-idle engine)."""
            e = tmps.tile([128, 1024], BF, tag="e")
            nc.scalar.activation(e[:], psum_ap, AF.Exp)
            r = tmps.tile([128, 1024], BF, tag="r")
            nc.scalar.activation(r[:], psum_ap, AF.Relu)
            t = tmps.tile([128, 1024], BF, tag="t")
            nc.vector.tensor_scalar_min(t[:], e[:], 1.0)
            nc.gpsimd.tensor_tensor(dst_ap, t[:], r[:], OP.add)

        def load_x(X, b):
            xblocks = [(0, 512), (512, 1536), (2048, 2048)]
            for (c0, cw) in xblocks:
                cs = slice(c0, c0 + cw)
                nc.sync.dma_start(out=X[:, :, cs], in_=x_d.ap()[b, :, :, cs])

        def qv_accum(pqv, i, qv_ps):
            # fold pqv chunk i into the qv accumulation (stage C)
            for qh in range(2):
                nc.tensor.matmul(
                    qv_ps[:, qh, 0:256],
                    pqv[:, i, :, qh * 128:qh * 128 + 128],
                    pqv[:, i, :, 256:512],
                    start=(i == 0), stop=(i == 15),
                    perf_mode=DR,
                )

        def b_span(X, pqv, i, qv_ps):
            # one stage-B span: phi(q^T), phi(v^T) chunk pair -> pqv[:, i].
            # The qv accumulation for chunk i-1 is emitted here (one span
            # later) so its poly has surely completed when PE reaches it --
            # no head-of-line blocking in the PE wait queue.
            ps = psmm.tile([128, 2, 512], FP, tag="mm")
            for j in range(2):
                nk = i * 2 + j
                nc.tensor.matmul(
                    ps[:, j, :],
                    X[:, :, nk * 128:(nk + 1) * 128],
                    wqv_sb[:],
                    start=True, stop=True,
                    perf_mode=DR,
                )
            poly_phi(flat2(ps[:]), flat2(pqv[:, i]))
            if i > 0:
                qv_accum(pqv, i - 1, qv_ps)

        def a_span(X, phik, si):
            # one stage-A span: phi_k block si (m = si//4, i = si%4)
            m, i = si // 4, si % 4
            ps = psmm.tile([128, 2, 512], FP, tag="mm")
            for j in range(2):
                n0 = (i * 2 + j) * 512
                nc.tensor.matmul(
                    ps[:, j, :],
                    wk_sb[:, :, m * 128:(m + 1) * 128],
                    X[:, :, n0:n0 + 512],
                    start=True, stop=True,
                    perf_mode=DR,
                )
            dst = phik[m][:, i * 1024:(i + 1) * 1024]
            # poly-assigned A-spans are scattered across the window (not
            # bunched at the tail) so consecutive DVE-consumed spans never
            # stall PSUM recycling long enough to starve ACT
            n_poly = 16 - n_schemeg
            poly_set = ((8, 12, 14), (8, 12, 14, 10), (8, 10, 12, 14, 9))[
                max(0, min(2, n_poly - 3))] if n_poly >= 3 else ()
            if state["aspan"] % 16 in poly_set[:n_poly]:
                poly_phi(flat2(ps[:]), dst)
            else:
                phi_schemeg(flat2(ps[:]), dst)
            state["aspan"] += 1

        def stage_C(qv_ps):
            # qv accumulation happened inline in b_span; here just the
            # qv copy + W2^T[d, o] = sum_c qv[c, d] WoT[c, o] chain
            # latency-critical copies run on DVE: at window end the ACT queue
            # has a multi-op backlog while DVE has just drained its polys
            qv_sb = smalls.tile([128, 2, 256], F32R, tag="qv_sb")
            nc.vector.tensor_scalar_add(qv_sb[:], qv_ps[:, :, 0:256], 0.0)

            w2_ps = psmm.tile([128, 2, 256], FP, tag="mm", name="w2ps")
            for dh in range(2):
                for cc in range(2):
                    nc.tensor.matmul(
                        w2_ps[:, dh, :],
                        qv_sb[:, cc, dh * 128:(dh + 1) * 128],
                        wo_sb[:, cc, :],
                        start=(cc == 0), stop=(cc == 1),
                    )
            w2_sb = smalls.tile([128, 2, 256], BF, tag="w2_sb")
            nc.vector.tensor_scalar_add(flat2(w2_sb[:]), flat2(w2_ps[:]), 0.0)
            return w2_sb

        def d_span(prev, si, ostage):
            b, w2_sb, phik = prev
            m, i = si // 4, si % 4
            ps = psmm.tile([128, 2, 512], FP, tag="mm")
            for j in range(2):
                n0 = (i * 2 + j) * 512
                for dd in range(2):
                    nc.tensor.matmul(
                        ps[:, j, :],
                        w2_sb[:, dd, m * 128:(m + 1) * 128],
                        phik[dd][:, n0:n0 + 512],
                        start=(dd == 0), stop=(dd == 1),
                    )
            o_sb = ostage[m]
            dst = o_sb[:, i * 1024:(i + 1) * 1024]
            if state["out"] % out_act_mod == out_act_mod - 1:
                nc.vector.tensor_scalar_add(dst, flat2(ps[:]), 0.0)
            else:
                nc.scalar.activation(dst, flat2(ps[:]), AF.Copy)
            state["out"] += 1
            if i % 2 == 1:
                # consolidated DMA per 2048-column half
                n0 = (i - 1) * 1024
                nc.sync.dma_start(
                    out=out_d.ap()[b, m, :, n0:n0 + 2048],
                    in_=o_sb[:, n0:n0 + 2048],
                )

        # ---- persistent ping-pong slots (explicit 2-deep pipeline) ----
        X_slots = [singles.tile([128, 2, HW], F8, tag=f"x{p}", name=f"xs{p}")
                   for p in range(2)]
        pqv_slots = [singles.tile([128, 16, 2, 512], F8, tag=f"pqv{p}", name=f"pqvs{p}")
                     for p in range(2)]
        phik_slots = [[singles.tile([128, HW], BF, tag=f"phik{p}_{m}", name=f"pks{p}_{m}")
                       for m in range(2)] for p in range(2)]
        qv_ps = psacc.tile([128, 2, 512], FP, tag="acc", name="qvps")

        def unit(t, prev):
            """One batch-unit: B+A window for batch t%2 with the previous
            unit's D-spans woven in, then the C/C2 chain."""
            bt = t % 2
            X, pqv, phik = X_slots[bt], pqv_slots[bt], phik_slots[bt]
            load_x(X, bt)
            ostage = None
            if prev is not None:
                ostage = [outp.tile([128, HW], BF, tag="osb", name=f"osb{t%4}_{m}")
                          for m in range(2)]
            # A-spans (depend only on X, never stall) at the window edges;
            # D-spans of the previous unit in the middle (i=4..11): late
            # enough that w2 is ready (no PE head-of-line blocking), early
            # enough that PE's heavier D-matmul load spreads across the window
            na = 0
            for i in range(16):
                b_span(X, pqv, i, qv_ps)
                if 4 <= i < 12 and prev is not None:
                    d_span(prev, i - 4, ostage)
                else:
                    if na < 8:
                        a_span(X, phik, na)
                        na += 1
            while na < 8:
                a_span(X, phik, na)
                na += 1
            qv_accum(pqv, 15, qv_ps)
            w2 = stage_C(qv_ps)
            return (bt, w2, phik)

        def drain_d(prev):
            ostage = [outp.tile([128, HW], BF, tag="osb", name=f"osbd_{m}")
                      for m in range(2)]
            for si in range(8):
                d_span(prev, si, ostage)

        def emit(n_bodies, _iv=None):
            state["out"] = 0
            state["aspan"] = 0
            prev = None
            for t in range(2 * n_bodies):
                prev = unit(t, prev)
            drain_d(prev)

        if repeat <= 4:
            emit(repeat)
        else:
            # unroll bodies inside the hardware loop: plain For_i has an
            # all-engine barrier per iteration (a full pipeline drain);
            # unrolling amortizes it, and the per-iteration D-drain tail
            # doubles as the loop-carried-dependence safety net
            unroll = 1
            for u in (8, 4, 2):
                if repeat % u == 0:
                    unroll = u
                    break
            with tc.For_i(0, repeat // unroll, 1) as iv:
                emit(unroll, iv)

    nc.compile()
    return nc


_nc_cache = {}


def _get_nc(repeat: int = 1):
    if repeat not in _nc_cache:
        _nc_cache[repeat] = build_kernel(repeat)
    return _nc_cache[repeat]


def make_in_maps(x, Wq, Wk, Wv, Wo, bo):
    import ml_dtypes

    f8 = np.dtype(ml_dtypes.float8_e4m3)
    # x: [B, C, H, W] -> per-core [NB, 128, 2, HW] fp8 (p-major, cc interleave)
    x8 = np.asarray(x, dtype=np.float32).reshape(B, 2, 128, HW).transpose(0, 2, 1, 3)
    x8 = np.ascontiguousarray(x8).astype(f8)
    # wqv[p, cc, 0:256] = Wq.T[cc*128+p, :], [...,256:512] = Wv.T
    wqt = np.asarray(Wq, dtype=np.float32).T.reshape(2, 128, 256)
    wvt = np.asarray(Wv, dtype=np.float32).T.reshape(2, 128, 256)
    wqv = np.concatenate([wqt, wvt], axis=2).transpose(1, 0, 2)
    wqv = np.ascontiguousarray(wqv).astype(f8)
    wkt = np.asarray(Wk, dtype=np.float32).T.reshape(2, 128, 256).transpose(1, 0, 2)
    wk8 = np.ascontiguousarray(wkt).astype(f8)
    wot = np.asarray(Wo, dtype=np.float32).T.reshape(2, 128, 256).transpose(1, 0, 2)
    wo32 = np.ascontiguousarray(wot)
    return [
        {"x": x8[i * NB:(i + 1) * NB], "wqv": wqv, "wk": wk8, "wo": wo32}
        for i in range(NCORES)
    ]


def kernel(x, Wq, Wk, Wv, Wo, bo):
    nc = _get_nc(repeat=1)
    in_maps = make_in_maps(x, Wq, Wk, Wv, Wo, bo)
    res = bass_utils.run_bass_kernel_spmd(nc, in_maps, core_ids=list(range(NCORES)))
    out = np.concatenate([res.results[i]["out"] for i in range(NCORES)], axis=0)
    out = out.astype(np.float32).reshape(B, C, H, W)
    out += np.asarray(bo, dtype=np.float32)[None, :, None, None]
    return np.ascontiguousarray(out)
